# revision 3
# baseline (speedup 1.0000x reference)
"""NTN kernel, f16-stream variant with software-pipelined epilogue.

y = relu(x1 @ M + c) @ u,  M = V[:,:D] + (W @ x2)^T  (128x16),
c = x2 @ V[:,D:]^T + b,    u = U[:,0].

relu(z+c) = max(z,-c) + c turns the affine tail into
    y[r] = sum_k u[k]*max(z[r,k], -c[k]) + sum_k u[k]*c[k]
so the kernel streams x1 (f16, 2 B/elem -> 16 MB/core), does ONE
128x128 @ 128x16 f16 matmul per row-tile, then per 32-tile group:
DVE max, Pool mult-by-u, DVE reduce over K.

Two scheduling rules keep the group pipeline at the Pool-mult rate
(~1.4us/group) instead of the serial max+mult+reduce rate (~2.8us):
  - reduce(g) is emitted AFTER max(g+2), so in DVE program order a
    slow mult never delays the next group's max (which is what
    releases the PSUM bank the matmuls need);
  - y is stored in 4-group slabs from the Pool engine (2 groups
    behind), so the final store isn't one serial 250 KB DMA tail.

Engines stay single-duty (a dma_start blocked on a tile slot stalls
every later instruction on its engine):
    SP/ACT: x DMA issue (alternating chunks)   PE: 1 matmul/tile
    DVE:    max + reduce            Pool: mul by u, param DMAs, y stores

Measured error ~5e-4 L2-relative vs the fp32 reference (tolerance 2e-2).
"""

import numpy as np

import concourse.bass as bass
import concourse.bacc as bacc
import concourse.mybir as mybir
import concourse.tile as tile

N, D, K = 500000, 128, 16
NCORES = 8
ROWS_PER_CORE = N // NCORES
TILES = 489
RPC = TILES * 128
GROUP = 32
DMA_CHUNK = 64
F32 = mybir.dt.float32
F16 = mybir.dt.float16


def _build_program():
    nc = bacc.Bacc(None, target_bir_lowering=False)

    xh = nc.dram_tensor("xh", [128, RPC], F16, kind="ExternalInput")
    mt = nc.dram_tensor("mt", [128, K], F16, kind="ExternalInput")
    negc = nc.dram_tensor("negc", [128, GROUP, K], F32, kind="ExternalInput")
    ub = nc.dram_tensor("ub", [128, GROUP, K], F32, kind="ExternalInput")
    y = nc.dram_tensor("y", [128, TILES], F32, kind="ExternalOutput")

    with tile.TileContext(nc) as tc:
        with (
            tc.tile_pool(name="singles", bufs=1) as singles,
            tc.tile_pool(name="xin", bufs=6) as xin,
            tc.tile_pool(name="zp", bufs=6, space="PSUM") as zpool,
            tc.tile_pool(name="work", bufs=6) as work,
            tc.tile_pool(name="yout", bufs=1) as yout,
        ):
            # Uniform big chunks; the final partial chunk is broken into
            # small pieces so that after the last byte lands only a small
            # group's matmul + epilogue remain on the critical path.
            sizes = []
            rem = TILES
            while rem > DMA_CHUNK:
                sizes.append(DMA_CHUNK)
                rem -= DMA_CHUNK
            while rem > 16:
                sizes.append(16)
                rem -= 16
            sizes.append(rem)

            # Alternate chunks between the two HWDGE queues (sync, scalar)
            # so both drain together.
            engs = (nc.sync, nc.scalar)
            chunk_tiles = []
            c0 = 0
            for i, nct in enumerate(sizes):
                xh_t = xin.tile([128, DMA_CHUNK * 128], F16, tag="xh")
                engs[i % 2].dma_start(
                    xh_t[:, : nct * 128], xh[:, c0 * 128 : (c0 + nct) * 128]
                )
                chunk_tiles.append((c0, nct, xh_t))
                c0 += nct
            assert c0 == TILES

            mt_sb = singles.tile([128, K], F16)
            nc.gpsimd.dma_start(mt_sb, mt[:, :])
            negc_sb = singles.tile([128, GROUP, K], F32)
            nc.gpsimd.dma_start(negc_sb, negc[:, :, :])
            ub_sb = singles.tile([128, GROUP, K], F32)
            nc.gpsimd.dma_start(ub_sb, ub[:, :, :])

            y_sb = yout.tile([128, TILES], F32)

            # (t0, nt, prod_tile) for groups whose reduce hasn't been
            # emitted yet; reduce is deferred 2 groups behind max.
            pending = []
            stored = 0          # tiles whose y slab has been stored
            reduced = 0         # tiles whose reduce has been emitted
            done_groups = 0

            def emit_reduce():
                nonlocal reduced, done_groups
                t0, nt, prod = pending.pop(0)
                nc.vector.tensor_reduce(
                    y_sb[:, t0 : t0 + nt], prod[:, :nt, :],
                    axis=mybir.AxisListType.X, op=mybir.AluOpType.add,
                )
                reduced = t0 + nt
                done_groups += 1

            def emit_ystore():
                nonlocal stored
                if reduced > stored:
                    nc.gpsimd.dma_start(
                        y[:, stored:reduced], y_sb[:, stored:reduced]
                    )
                    stored = reduced

            for c0, nct, xh_t in chunk_tiles:
                g0 = 0
                while g0 < nct:
                    nt = min(GROUP, nct - g0)
                    t0 = c0 + g0
                    zp = zpool.tile([128, GROUP, K], F32, tag="z")
                    for t in range(nt):
                        sl = slice((g0 + t) * 128, (g0 + t + 1) * 128)
                        nc.tensor.matmul(
                            zp[:, t, :], xh_t[:, sl], mt_sb[:, :],
                            start=True, stop=True,
                        )
                    relu = work.tile([128, GROUP, K], F32, tag="relu")
                    nc.vector.tensor_tensor(
                        relu[:, :nt, :], zp[:, :nt, :], negc_sb[:, :nt, :],
                        op=mybir.AluOpType.max,
                    )
                    prod = work.tile([128, GROUP, K], F32, tag="prod")
                    nc.gpsimd.tensor_tensor(
                        prod[:, :nt, :], relu[:, :nt, :], ub_sb[:, :nt, :],
                        op=mybir.AluOpType.mult,
                    )
                    pending.append((t0, nt, prod))
                    if len(pending) > 2:
                        emit_reduce()
                        if done_groups % 4 == 0:
                            emit_ystore()
                    g0 += nt

            while pending:
                emit_reduce()
            emit_ystore()

    nc.compile()
    return nc


_NC_CACHE = None


def _get_program():
    global _NC_CACHE
    if _NC_CACHE is None:
        _NC_CACHE = _build_program()
    return _NC_CACHE


def _host_prep(x1, x2, V, W, b, U):
    x1 = np.asarray(x1, dtype=np.float32)
    x2 = np.asarray(x2, dtype=np.float64)
    V = np.asarray(V, dtype=np.float64)
    W = np.asarray(W, dtype=np.float64)
    b = np.asarray(b, dtype=np.float64)
    U = np.asarray(U, dtype=np.float64)

    M = V[:, :D] + np.einsum("kde,e->kd", W, x2[0])
    c = (x2[0] @ V[:, D:].T) + b
    u = U[:, 0]
    const = float(np.dot(u, c))

    mt = np.ascontiguousarray(M.astype(np.float16).T)
    negc_t = np.broadcast_to(
        (-c).astype(np.float32), (128, GROUP, K)
    ).copy()
    ub_t = np.broadcast_to(u.astype(np.float32), (128, GROUP, K)).copy()

    in_maps = []
    for cidx in range(NCORES):
        sl = x1[cidx * ROWS_PER_CORE : (cidx + 1) * ROWS_PER_CORE]
        hbuf = np.zeros((128, RPC), dtype=np.float16)
        hbuf[:, :ROWS_PER_CORE] = sl.T.astype(np.float16)
        in_maps.append({"xh": hbuf, "mt": mt, "negc": negc_t, "ub": ub_t})
    return in_maps, const


def _gather(results, const):
    outs = []
    for cidx in range(NCORES):
        yc = np.asarray(results[cidx]["y"])
        outs.append(yc.T.reshape(-1)[:ROWS_PER_CORE])
    yfull = np.concatenate(outs) + np.float32(const)
    return yfull.reshape(N, 1).astype(np.float32)


def run_device(in_maps, trace=False):
    from concourse.bass_utils import run_bass_kernel_spmd

    nc = _get_program()
    res = run_bass_kernel_spmd(
        nc, in_maps, core_ids=list(range(NCORES)), trace=trace
    )
    return res


def kernel(x1, x2, V, W, b, U):
    in_maps, const = _host_prep(x1, x2, V, W, b, U)
    res = run_device(in_maps, trace=False)
    return _gather(res.results, const)


# revision 4
# speedup vs baseline: 1.0816x; 1.0816x over previous
"""NTN kernel, f16-stream variant with u folded into the matmul.

y = relu(x1 @ M + c) @ u,  M = V[:,:D] + (W @ x2)^T  (128x16),
c = x2 @ V[:,D:]^T + b,    u = U[:,0].

Folding u into M removes the per-group multiply (which, on the Pool
engine, was the pipeline's rate limiter and produced a ~11us serial
tail).  With s = sign(u):

    u_k * relu(z_k + c_k) = s_k * max(|u_k| z_k, -|u_k| c_k) + u_k c_k

Host prep permutes columns so positive-u columns come first (KP of
them) and pre-negates the negative-u columns of M' and c', turning
s_k * max(...) into:

    k < KP:  max(z'_k, -c'_k)         (z' = x @ M'>0 cols)
    k >= KP: min(z''_k, +c'_k)        (z'' = x @ -M' cols)

so y[r] = plain_sum_k(e[r,k]) + sum_k u_k c_k, where e is one DVE max
pass over the first KP columns, one DVE min pass over the rest, and
one DVE reduce — no multiply, no sign fixup.  KP depends on the input
u, so the program is built after inputs arrive (cached per KP).

The kernel streams x1 (f16, 2 B/elem -> 16 MB/core; rel err ~5e-4,
tolerance 2e-2), one 128x128 @ 128x16 f16 matmul per row-tile.
Scheduling details that keep the group pipeline at the DMA rate:
  - reduce(g) is emitted AFTER max/min(g+2), so in DVE program order
    nothing delays the next group's max (which releases the PSUM bank
    the matmuls need);
  - y is stored in 4-group slabs from the Pool engine (2 groups
    behind), so the final store isn't one serial 250 KB DMA tail.

Engines stay single-duty (a dma_start blocked on a tile slot stalls
every later instruction on its engine):
    SP/ACT: x DMA issue (alternating chunks)   PE: 1 matmul/tile
    DVE:    max/min + reduce            Pool: param DMAs, y stores
"""

import numpy as np

import concourse.bass as bass
import concourse.bacc as bacc
import concourse.mybir as mybir
import concourse.tile as tile

N, D, K = 500000, 128, 16
NCORES = 8
ROWS_PER_CORE = N // NCORES
TILES = 489
RPC = TILES * 128
GROUP = 32
DMA_CHUNK = 64
F32 = mybir.dt.float32
F16 = mybir.dt.float16


def _build_program(kp):
    nc = bacc.Bacc(None, target_bir_lowering=False)

    xh = nc.dram_tensor("xh", [128, RPC], F16, kind="ExternalInput")
    mt = nc.dram_tensor("mt", [128, K], F16, kind="ExternalInput")
    cb = nc.dram_tensor("cb", [128, GROUP, K], F32, kind="ExternalInput")
    y = nc.dram_tensor("y", [128, TILES], F32, kind="ExternalOutput")

    with tile.TileContext(nc) as tc:
        with (
            tc.tile_pool(name="singles", bufs=1) as singles,
            tc.tile_pool(name="xin", bufs=6) as xin,
            tc.tile_pool(name="zp", bufs=6, space="PSUM") as zpool,
            tc.tile_pool(name="work", bufs=6) as work,
            tc.tile_pool(name="yout", bufs=1) as yout,
        ):
            # Uniform big chunks; the final partial chunk is broken into
            # small pieces so that after the last byte lands only a small
            # group's matmul + epilogue remain on the critical path.
            sizes = []
            rem = TILES
            while rem > DMA_CHUNK:
                sizes.append(DMA_CHUNK)
                rem -= DMA_CHUNK
            while rem > 16:
                sizes.append(16)
                rem -= 16
            sizes.append(rem)

            # Alternate chunks between the two HWDGE queues (sync, scalar)
            # so both drain together.
            engs = (nc.sync, nc.scalar)
            chunk_tiles = []
            c0 = 0
            for i, nct in enumerate(sizes):
                xh_t = xin.tile([128, DMA_CHUNK * 128], F16, tag="xh")
                engs[i % 2].dma_start(
                    xh_t[:, : nct * 128], xh[:, c0 * 128 : (c0 + nct) * 128]
                )
                chunk_tiles.append((c0, nct, xh_t))
                c0 += nct
            assert c0 == TILES

            mt_sb = singles.tile([128, K], F16)
            nc.gpsimd.dma_start(mt_sb, mt[:, :])
            cb_sb = singles.tile([128, GROUP, K], F32)
            nc.gpsimd.dma_start(cb_sb, cb[:, :, :])

            y_sb = yout.tile([128, TILES], F32)

            # (t0, nt, elem_tile) for groups whose reduce hasn't been
            # emitted yet; reduce is deferred 2 groups behind max/min.
            pending = []
            stored = 0          # tiles whose y slab has been stored
            reduced = 0         # tiles whose reduce has been emitted
            done_groups = 0

            def emit_reduce():
                nonlocal reduced, done_groups
                t0, nt, elem = pending.pop(0)
                nc.vector.tensor_reduce(
                    y_sb[:, t0 : t0 + nt], elem[:, :nt, :],
                    axis=mybir.AxisListType.X, op=mybir.AluOpType.add,
                )
                reduced = t0 + nt
                done_groups += 1

            def emit_ystore():
                nonlocal stored
                if reduced > stored:
                    nc.gpsimd.dma_start(
                        y[:, stored:reduced], y_sb[:, stored:reduced]
                    )
                    stored = reduced

            for c0, nct, xh_t in chunk_tiles:
                g0 = 0
                while g0 < nct:
                    nt = min(GROUP, nct - g0)
                    t0 = c0 + g0
                    zp = zpool.tile([128, GROUP, K], F32, tag="z")
                    for t in range(nt):
                        sl = slice((g0 + t) * 128, (g0 + t + 1) * 128)
                        nc.tensor.matmul(
                            zp[:, t, :], xh_t[:, sl], mt_sb[:, :],
                            start=True, stop=True,
                        )
                    elem = work.tile([128, GROUP, K], F32, tag="elem")
                    if kp > 0:
                        nc.vector.tensor_tensor(
                            elem[:, :nt, :kp], zp[:, :nt, :kp],
                            cb_sb[:, :nt, :kp], op=mybir.AluOpType.max,
                        )
                    if kp < K:
                        nc.vector.tensor_tensor(
                            elem[:, :nt, kp:], zp[:, :nt, kp:],
                            cb_sb[:, :nt, kp:], op=mybir.AluOpType.min,
                        )
                    pending.append((t0, nt, elem))
                    if len(pending) > 2:
                        emit_reduce()
                        if done_groups % 4 == 0:
                            emit_ystore()
                    g0 += nt

            while pending:
                emit_reduce()
            emit_ystore()

    nc.compile()
    return nc


_NC_CACHE = {}


def _get_program(kp):
    if kp not in _NC_CACHE:
        _NC_CACHE[kp] = _build_program(kp)
    return _NC_CACHE[kp]


def _host_prep(x1, x2, V, W, b, U):
    x1 = np.asarray(x1, dtype=np.float32)
    x2 = np.asarray(x2, dtype=np.float64)
    V = np.asarray(V, dtype=np.float64)
    W = np.asarray(W, dtype=np.float64)
    b = np.asarray(b, dtype=np.float64)
    U = np.asarray(U, dtype=np.float64)

    M = V[:, :D] + np.einsum("kde,e->kd", W, x2[0])   # (K, D)
    c = (x2[0] @ V[:, D:].T) + b                      # (K,)
    u = U[:, 0]                                       # (K,)
    const = float(np.dot(u, c))

    # Positive-u columns first; fold |u| into M and c; negate the
    # negative-u columns of M so the epilogue is max / min / plain sum.
    perm = np.argsort(u <= 0, kind="stable")
    kp = int(np.sum(u > 0))
    up = u[perm]
    Mp = M[perm] * up[:, None]          # rows still (K, D); sign included
    cp = c[perm] * np.abs(up)
    # column k < kp:  z' = x@(|u|M),  compare vs -c'  -> max(z', -c')
    # column k >= kp: z''= x@(u M) (already negative-scaled), vs +c'
    cmpv = np.where(np.arange(K) < kp, -cp, cp)

    mt = np.ascontiguousarray(Mp.T.astype(np.float16))
    cb = np.broadcast_to(cmpv.astype(np.float32), (128, GROUP, K)).copy()

    in_maps = []
    for cidx in range(NCORES):
        sl = x1[cidx * ROWS_PER_CORE : (cidx + 1) * ROWS_PER_CORE]
        hbuf = np.zeros((128, RPC), dtype=np.float16)
        hbuf[:, :ROWS_PER_CORE] = sl.T.astype(np.float16)
        in_maps.append({"xh": hbuf, "mt": mt, "cb": cb})
    return in_maps, const, kp


def _gather(results, const):
    outs = []
    for cidx in range(NCORES):
        yc = np.asarray(results[cidx]["y"])
        outs.append(yc.T.reshape(-1)[:ROWS_PER_CORE])
    yfull = np.concatenate(outs) + np.float32(const)
    return yfull.reshape(N, 1).astype(np.float32)


def run_device(in_maps, kp, trace=False):
    from concourse.bass_utils import run_bass_kernel_spmd

    nc = _get_program(kp)
    res = run_bass_kernel_spmd(
        nc, in_maps, core_ids=list(range(NCORES)), trace=trace
    )
    return res


def kernel(x1, x2, V, W, b, U):
    in_maps, const, kp = _host_prep(x1, x2, V, W, b, U)
    res = run_device(in_maps, kp, trace=False)
    return _gather(res.results, const)
